# revision 14
# baseline (speedup 1.0000x reference)
"""GAT message-passing kernel for TRN2 (8 NeuronCores, SPMD).

Algorithm (matches the jax reference up to a softmax shift, which cancels):
  proj = src @ W_src.T ; s_src[n,h] = src[n].w_s[h] ; s_trg[n,h] = trg[n].w_t[h]
  score_e = leakyrelu(s_src[si]+s_trg[ti]) ; p_e = exp(score_e - C_OFF)
  out[t,h,:] = sum_{e: ti=t} p_e * proj[si_e,h,:] / (sum p_e + eps)

Sharding: edges sorted by target; core c owns targets [c*TPC,(c+1)*TPC).
Per 128-target window a one-hot matmul segment-sums numerator||denominator
into PSUM.  Per-edge rows (proj bf16 | s_src f32, 288B payload in a 512B
row) come from a partition-major node table built on device in P0 and read
back via dma_gather.  Windows are processed in groups; within a group the
gather-order is slab-major so each (group, slab) pair needs only one
dma_gather call (994ns fixed cost per call).  All bulk HWDGE transfers use
partition-major DRAM layouts so descriptors are large and few.
"""
import os
import numpy as np
import ml_dtypes

import concourse.bacc as bacc
import concourse.mybir as mybir
import concourse.tile as tile
from concourse.bass_utils import run_bass_kernel_spmd

BF16 = mybir.dt.bfloat16
F32 = mybir.dt.float32
I16 = mybir.dt.int16

NH, FOUT, D = 8, 16, 128
HF = NH * FOUT  # 128
NEG_SLOPE = 0.2
C_OFF = 16.0
SLAB = 32768       # table rows per gather slab (int16 index limit)
ROW = 256          # bf16 slots per node-table row (512B); payload = 144
WG = 2             # windows per group
MAX_CT = 28        # max tiles (x128 idx) per dma_gather call
NQ = 4             # SWDGE queues

LAST_EXEC_NS = None


def _install_trace_shim():
    """Register the axon NTFF profile hook (missing antenv.axon_hooks shim)."""
    import sys
    import types

    if "antenv.axon_hooks" in sys.modules:
        return True
    try:
        mod = types.ModuleType("antenv.axon_hooks")
        mod._hook = None
        mod.set_axon_ntff_profile_hook = lambda h: setattr(mod, "_hook", h)
        mod.get_axon_ntff_profile_hook = lambda: mod._hook
        from trn_agent_boot.trn_boot import _ntff_profile_via_ctypes

        mod._hook = _ntff_profile_via_ctypes("/opt/axon/libaxon_pjrt.so")
        sys.modules["antenv.axon_hooks"] = mod
        import concourse.bass_utils as bu

        bu.upload_artifacts = lambda tmpdir: tmpdir
        return True
    except Exception:
        return False


def _wrap_idx(v):
    """[ct*128] int array -> [128, ct*8] int16 wrapped+replicated layout."""
    w = np.asarray(v, dtype=np.int16).reshape(-1, 16).T  # [16, ct*8]
    return np.tile(w, (8, 1))


def build_schedule(si, ti, N, ncores):
    """Shared SPMD schedule (identical across cores) + per-core index arrays.

    Table row of node n is (n%128)*NT0 + n//128 (partition-major table).
    Tile order (gather order): for each group of WG windows, slab-major:
      [(s0: w0 tiles, w1 tiles, ...), (s1: ...), ...]
    so one dma_gather call covers a whole (group, slab) run.
    Window-local order: window w's tiles = concat over slabs (ascending).
    """
    TPC = N // ncores
    WPC = (TPC + 127) // 128
    NPAD = ((N + 127) // 128) * 128
    NT0 = NPAD // 128
    nslabs = (NPAD + SLAB - 1) // SLAB

    si = np.asarray(si, dtype=np.int64)
    ti = np.asarray(ti, dtype=np.int64)
    row = (si % 128) * NT0 + si // 128
    s_of = row // SLAB
    ri = row % SLAB
    core = ti // TPC
    tloc = ti - core * TPC
    w_of = tloc >> 7
    tl_of = tloc & 127

    counts = np.zeros((ncores, WPC, nslabs), dtype=np.int64)
    np.add.at(counts, (core, w_of, s_of), 1)
    NT = np.ceil(counts.max(axis=0) / 128).astype(np.int64)  # [WPC, nslabs]
    NW = NT.sum(axis=1)                                      # [WPC]

    groups = [(g, min(g + WG, WPC)) for g in range(0, WPC, WG)]

    # gather-order tile offsets
    goff = {}
    off = 0
    group_t0 = []
    for (w0, w1) in groups:
        group_t0.append(off)
        for s in range(nslabs):
            for w in range(w0, w1):
                goff[(w, s)] = off
                off += int(NT[w, s])
    T_total = off
    group_t0.append(off)

    # window-order tile offsets
    woff = np.zeros(WPC + 1, dtype=np.int64)
    woff[1:] = np.cumsum(NW)

    # per-window runs: (window-local j0, gather-order tile o0, ntiles)
    runs = []
    for w in range(WPC):
        r = []
        j0 = 0
        for s in range(nslabs):
            nt = int(NT[w, s])
            if nt:
                r.append((j0, goff[(w, s)], nt))
                j0 += nt
        runs.append(r)

    # gather calls: per group, per slab, split by MAX_CT
    # (slab s tiles of a group are contiguous in gather order)
    calls = []       # per group: list of (slab, o0_global, ct, idxcol)
    gidx = []        # per group: (idxcol0, idxcol1)
    idxcol = 0
    for gi, (w0, w1) in enumerate(groups):
        cl = []
        c0 = idxcol
        for s in range(nslabs):
            o0 = goff[(w0, s)]
            ct_all = sum(int(NT[w, s]) for w in range(w0, w1))
            k = 0
            while k < ct_all:
                ct = min(MAX_CT, ct_all - k)
                cl.append((s, o0 + k, ct, idxcol))
                idxcol += ct * 8
                k += ct
        calls.append(cl)
        gidx.append((c0, idxcol))
    idx_cols = idxcol

    # per-core index arrays
    per_core = []
    for c in range(ncores):
        m = core == c
        cri, cs, cw, ctl = ri[m], s_of[m], w_of[m], tl_of[m]
        order = np.lexsort((cs, cw))
        cri, cs, cw, ctl = (a[order] for a in (cri, cs, cw, ctl))
        keys = cw * nslabs + cs
        starts = np.searchsorted(keys, np.arange(WPC * nslabs))
        ends = np.searchsorted(keys, np.arange(WPC * nslabs), side="right")

        ri_t = np.zeros((T_total, 128), dtype=np.int64)
        tl_t = np.full((T_total, 128), 255, dtype=np.int64)
        for w in range(WPC):
            for s in range(nslabs):
                a, b = starts[w * nslabs + s], ends[w * nslabs + s]
                cnt = b - a
                if cnt:
                    t0 = goff[(w, s)]
                    flat_r = ri_t[t0 : t0 + int(NT[w, s])].reshape(-1)
                    flat_t = tl_t[t0 : t0 + int(NT[w, s])].reshape(-1)
                    flat_r[:cnt] = cri[a:b]
                    flat_t[:cnt] = ctl[a:b]

        blocks = []
        for cl in calls:
            for (s, o0, ct, co) in cl:
                blocks.append(_wrap_idx(ri_t[o0 : o0 + ct].reshape(-1)))
        idx1 = (np.concatenate(blocks, axis=1) if blocks
                else np.zeros((128, 8), np.int16))

        # window-order tl (for oh build) and its x128 replication (for ohT)
        worder = np.concatenate(
            [np.arange(o0, o0 + nt) for w in range(WPC) for (j0, o0, nt) in runs[w]]
        ) if T_total else np.zeros(0, np.int64)
        tl_w = tl_t[worder]                      # [T,128] window-ordered
        tl2 = tl_w.T.astype(np.float32).astype(ml_dtypes.bfloat16)  # [128,T]
        wtl = tl_w.reshape(1, -1).astype(np.float32).astype(ml_dtypes.bfloat16)
        tlrep = np.broadcast_to(wtl, (128, T_total * 128)).copy()
        per_core.append((idx1, tl2, tlrep))

    sched = dict(TPC=TPC, WPC=WPC, NPAD=NPAD, NT0=NT0, nslabs=nslabs,
                 NT=NT, NW=NW, T_total=T_total, groups=groups,
                 group_t0=group_t0, woff=woff, runs=runs, calls=calls,
                 gidx=gidx, idx_cols=idx_cols)
    return sched, per_core


def build_nc(sched):
    WPC, NPAD, NT0 = sched["WPC"], sched["NPAD"], sched["NT0"]
    NW, T_total = sched["NW"], sched["T_total"]
    groups, group_t0 = sched["groups"], sched["group_t0"]
    woff, runs, calls = sched["woff"], sched["runs"], sched["calls"]
    gidx = sched["gidx"]
    idx_cols = sched["idx_cols"]
    nslabs = sched["nslabs"]
    NWmax = int(max(NW)) if len(NW) else 1
    TG = max(group_t0[i + 1] - group_t0[i] for i in range(len(groups)))
    GIC = max((c1 - c0) for (c0, c1) in gidx) if gidx else 8

    nc = bacc.Bacc("TRN2", target_bir_lowering=False, num_swdge_queues=NQ)
    srcT = nc.declare_dram_parameter("srcT", [128, NPAD], BF16, isOutput=False)
    trgTl = nc.declare_dram_parameter("trgTl", [128, WPC * 128], BF16, isOutput=False)
    wext = nc.declare_dram_parameter("wext", [128, 144], BF16, isOutput=False)
    iota = nc.declare_dram_parameter("iota", [128, 128], BF16, isOutput=False)
    iotac = nc.declare_dram_parameter("iotac", [128, 1], BF16, isOutput=False)
    idx1 = nc.declare_dram_parameter("idx1", [128, max(idx_cols, 8)], I16, isOutput=False)
    tl2p = nc.declare_dram_parameter("tl2", [128, max(T_total, 1)], BF16, isOutput=False)
    tlrep = nc.declare_dram_parameter("tlrep", [128, max(T_total * 128, 128)], BF16,
                                      isOutput=False)
    outp = nc.declare_dram_parameter("out", [128, WPC * 128], F32, isOutput=True)
    table = nc.dram_tensor("table", [NPAD, ROW], BF16)

    qrr = [0]

    def next_q():
        q = qrr[0]
        qrr[0] = (q + 1) % NQ
        return q

    with tile.TileContext(nc) as tc:
        with tc.tile_pool(name="const", bufs=1) as cp:
            wext_sb = cp.tile([128, 144], BF16)
            nc.sync.dma_start(out=wext_sb[:], in_=wext[:, :])
            iota_sb = cp.tile([128, 128], BF16)
            nc.sync.dma_start(out=iota_sb[:], in_=iota[:, :])
            iotac_sb = cp.tile([128, 1], BF16)
            nc.sync.dma_start(out=iotac_sb[:], in_=iotac[:, :])
            tl2_sb = cp.tile([128, max(T_total, 1)], BF16)
            nc.sync.dma_start(out=tl2_sb[:], in_=tl2p[:, :])
            cbias = cp.tile([128, 1], F32)
            nc.vector.memset(cbias[:], -C_OFF)
            iotaP = cp.tile([128, TG * 128], BF16)
            nc.vector.tensor_copy(
                out=iotaP[:],
                in_=iotac_sb[:, 0:1].to_broadcast([128, TG * 128]))
            strg_sb = cp.tile([128, WPC, 16], BF16)

            # ---- PE warm-up: dense matmul burst to flip HAM to K=8/8 ----
            with tc.tile_pool(name="wps0", bufs=1, space="PSUM") as wps0:
                wps = wps0.tile([128, 128], F32)
                for i in range(40):
                    nc.tensor.matmul(out=wps[:], lhsT=iota_sb[:], rhs=iota_sb[:],
                                     start=(i == 0), stop=(i == 39))

            # ---- P0b: core-local s_trg table, SBUF-resident ----
            with (
                tc.tile_pool(name="pbin", bufs=2) as pbin,
                tc.tile_pool(name="pblo", bufs=3) as pblo,
                tc.tile_pool(name="pbps", bufs=3, space="PSUM") as pbps,
            ):
                KCB = 49
                j = 0
                while j < WPC:
                    kc = min(KCB, WPC - j)
                    t_t = pbin.tile([128, KCB * 128], BF16, tag="t", name=f"t{j}")
                    nc.sync.dma_start(out=t_t[:, 0 : kc * 128],
                                      in_=trgTl[:, j * 128 : (j + kc) * 128])
                    b = 0
                    while b < kc:
                        g = min(3, kc - b)
                        psB = pbps.tile([128, 3 * 8], F32, tag="psB", name=f"pb{j}_{b}")
                        for k in range(g):
                            nc.tensor.matmul(
                                out=psB[:, k * 8 : (k + 1) * 8],
                                lhsT=t_t[:, (b + k) * 128 : (b + k + 1) * 128],
                                rhs=wext_sb[:, 136:144],
                                start=True, stop=True,
                            )
                        psB_r = psB[:, 0 : g * 8].rearrange("p (k c) -> p k c", c=8)
                        sl = strg_sb[:, j + b : j + b + g, :]
                        nc.vector.tensor_copy(out=sl[:, :, 0:8], in_=psB_r)
                        lob = pblo.tile([128, 3, 8], F32, tag="lo", name=f"lo{j}_{b}")
                        nc.vector.tensor_tensor(
                            out=lob[:, 0:g, :], in0=psB_r, in1=sl[:, :, 0:8],
                            op=mybir.AluOpType.subtract,
                        )
                        nc.scalar.copy(out=sl[:, :, 8:16], in_=lob[:, 0:g, :])
                        b += g
                    j += kc

            # ---- P0: packed node table (proj bf16 | s_src f32 | junk pad) ----
            table_r = table[0 : 128 * NT0, :].rearrange("(p k) c -> p k c", k=NT0)
            with (
                tc.tile_pool(name="p0in", bufs=2) as p0in,
                tc.tile_pool(name="p0row", bufs=2) as p0row,
                tc.tile_pool(name="p0ps", bufs=4, space="PSUM") as p0ps,
            ):
                KC = 24
                j = 0
                while j < NT0:
                    kc = min(KC, NT0 - j)
                    s_t = p0in.tile([128, KC * 128], BF16, tag="s", name=f"s{j}")
                    nc.sync.dma_start(out=s_t[:, 0 : kc * 128],
                                      in_=srcT[:, j * 128 : (j + kc) * 128])
                    row = p0row.tile([128, KC, ROW], BF16, tag="row", name=f"r{j}")
                    row_f32 = row[:].bitcast(F32)  # [128, KC, 128]
                    b = 0
                    while b < kc:
                        g = min(3, kc - b)
                        psA = p0ps.tile([128, 3 * 136], F32, tag="psA", name=f"pa{j}_{b}")
                        for k in range(g):
                            nc.tensor.matmul(
                                out=psA[:, k * 136 : (k + 1) * 136],
                                lhsT=s_t[:, (b + k) * 128 : (b + k + 1) * 128],
                                rhs=wext_sb[:, 0:136],
                                start=True, stop=True,
                            )
                        psA_r = psA[:, 0 : g * 136].rearrange("p (k c) -> p k c", c=136)
                        nc.scalar.copy(out=row[:, b : b + g, 0:HF],
                                       in_=psA_r[:, :, 0:HF])
                        nc.vector.tensor_copy(out=row_f32[:, b : b + g, 64:72],
                                              in_=psA_r[:, :, 128:136])
                        b += g
                    nc.sync.dma_start(out=table_r[:, j : j + kc, :],
                                      in_=row[:, 0:kc, :])
                    j += kc

            # ---- warm-up 2: keep PE busy across the P0 -> P1 gather gap ----
            with tc.tile_pool(name="wps1", bufs=1, space="PSUM") as wps1:
                wps = wps1.tile([128, 128], F32)
                for i in range(30):
                    nc.tensor.matmul(out=wps[:], lhsT=iota_sb[:], rhs=iota_sb[:],
                                     start=(i == 0), stop=(i == 29))

            # ---- P1: edge pass, grouped windows ----
            with (
                tc.tile_pool(name="g1p", bufs=2) as g1p,
                tc.tile_pool(name="trp", bufs=2) as trp,
                tc.tile_pool(name="idxp", bufs=2) as idxp,
                tc.tile_pool(name="ohtp", bufs=3) as ohtp,
                tc.tile_pool(name="ps2p", bufs=4, space="PSUM") as ps2p,
                tc.tile_pool(name="stsp", bufs=2) as stsp,
                tc.tile_pool(name="step", bufs=3) as step,
                tc.tile_pool(name="scp", bufs=3) as scp,
                tc.tile_pool(name="whp", bufs=3) as whp,
                tc.tile_pool(name="ohp", bufs=3) as ohp,
                tc.tile_pool(name="pswp", bufs=4, space="PSUM") as pswp,
                tc.tile_pool(name="epi", bufs=3) as epi,
                tc.tile_pool(name="osp", bufs=2) as osp,
            ):
                for gi, (w0, w1) in enumerate(groups):
                    gt0 = group_t0[gi]
                    gt = group_t0[gi + 1] - gt0
                    c0, c1 = gidx[gi]
                    idxt = idxp.tile([128, GIC], I16, tag="idx", name=f"ix{gi}")
                    nc.sync.dma_start(out=idxt[:, 0 : c1 - c0], in_=idx1[:, c0:c1])
                    G1 = g1p.tile([128, TG, ROW], BF16, tag="g1", name=f"g1_{gi}")
                    G1f = G1[:].bitcast(F32)  # [128, TG, 128]
                    for (s, o0, ct, co) in calls[gi]:
                        sb = s * SLAB
                        se = min(sb + SLAB, NPAD)
                        nc.gpsimd.dma_gather(
                            G1[:, o0 - gt0 : o0 - gt0 + ct, :],
                            table[sb:se, :],
                            idxt[:, co - c0 : co - c0 + ct * 8],
                            ct * 128, ct * 128, ROW,
                            queue_num=next_q(), single_packet=False,
                        )
                    tr = trp.tile([128, TG * 128], BF16, tag="tr", name=f"tr{gi}")
                    gw0 = int(woff[w0])
                    nc.sync.dma_start(
                        out=tr[:, 0 : gt * 128],
                        in_=tlrep[:, gw0 * 128 : (gw0 + gt) * 128],
                    )
                    outstage = osp.tile([128, WG * HF], F32, tag="os", name=f"os{gi}")
                    # --- group-level s_trg expansion: ohT_g, pse2 blocks,
                    #     one xbar transpose for the whole group ---
                    ohT = ohtp.tile([128, TG * 128], BF16, tag="ohT",
                                    name=f"ohT{gi}")
                    nc.vector.tensor_tensor(
                        out=ohT[:, 0 : gt * 128],
                        in0=tr[:, 0 : gt * 128],
                        in1=iotaP[:, 0 : gt * 128],
                        op=mybir.AluOpType.is_equal,
                    )
                    sts = stsp.tile([16, TG * 128], BF16, tag="sts", name=f"st{gi}")
                    for w in range(w0, w1):
                        nw = int(NW[w])
                        if nw == 0:
                            continue
                        jb = int(woff[w]) - gw0
                        nblk = (nw * 128 + 511) // 512
                        for bi in range(nblk):
                            x0 = jb * 128 + bi * 512
                            x1 = min((jb + nw) * 128, x0 + 512)
                            ps2 = ps2p.tile([16, 512], F32, tag="ps2",
                                            name=f"p2_{w}_{bi}")
                            nc.tensor.matmul(
                                out=ps2[:, 0 : x1 - x0],
                                lhsT=strg_sb[:, w, :],
                                rhs=ohT[:, x0:x1], start=True, stop=True,
                            )
                            nc.scalar.copy(out=sts[:, x0:x1], in_=ps2[:, 0 : x1 - x0])
                    ste = step.tile([128, TG, 16], BF16, tag="ste", name=f"se{gi}")
                    nc.sync.dma_start(out=ste[:, 0:gt, :],
                                      in_=sts[:, 0 : gt * 128], transpose=True)
                    for w in range(w0, w1):
                        nw = int(NW[w])
                        oslice = outstage[:, (w - w0) * HF : (w - w0 + 1) * HF]
                        if nw == 0:
                            nc.vector.memset(oslice, 0.0)
                            continue
                        jb = int(woff[w]) - gw0
                        sc = scp.tile([128, NWmax, 8], F32, tag="sc", name=f"sc{w}")
                        for (j0, o0, nt) in runs[w]:
                            nc.vector.tensor_tensor(
                                out=sc[:, j0 : j0 + nt, :],
                                in0=G1f[:, o0 - gt0 : o0 - gt0 + nt, 64:72],
                                in1=ste[:, jb + j0 : jb + j0 + nt, 0:8],
                                op=mybir.AluOpType.add,
                            )
                        nc.vector.tensor_tensor(
                            out=sc[:, 0:nw, :], in0=sc[:, 0:nw, :],
                            in1=ste[:, jb : jb + nw, 8:16], op=mybir.AluOpType.add,
                        )
                        lr = scp.tile([128, NWmax, 8], F32, tag="lr", name=f"lr{w}")
                        nc.scalar.activation(
                            lr[:, 0:nw, :], sc[:, 0:nw, :],
                            mybir.ActivationFunctionType.Prelu, alpha=NEG_SLOPE,
                        )
                        e1 = scp.tile([128, NWmax, 8], BF16, tag="e1", name=f"e1_{w}")
                        nc.scalar.activation(
                            e1[:, 0:nw, :], lr[:, 0:nw, :],
                            mybir.ActivationFunctionType.Exp, bias=cbias[:, 0:1],
                        )
                        wt = whp.tile([128, NWmax, 136], BF16, tag="wt", name=f"wt{w}")
                        nc.vector.tensor_copy(out=wt[:, 0:nw, 128:136],
                                              in_=e1[:, 0:nw, :])
                        for (j0, o0, nt) in runs[w]:
                            e_b = wt[:, j0 : j0 + nt, 128:136].rearrange(
                                "p w (h o) -> p w h o", o=1
                            ).to_broadcast([128, nt, 8, 16])
                            nc.vector.tensor_tensor(
                                out=wt[:, j0 : j0 + nt, 0:128].rearrange(
                                    "p w (h f) -> p w h f", f=16),
                                in0=G1[:, o0 - gt0 : o0 - gt0 + nt, 0:128].rearrange(
                                    "p w (h f) -> p w h f", f=16),
                                in1=e_b, op=mybir.AluOpType.mult,
                            )
                        oh = ohp.tile([128, NWmax * 128], BF16, tag="oh", name=f"oh{w}")
                        iota_b = iota_sb[:].rearrange(
                            "p (o c) -> p o c", o=1).to_broadcast([128, nw, 128])
                        tl_b = tl2_sb[:, int(woff[w]) : int(woff[w]) + nw].rearrange(
                            "p (w o) -> p w o", o=1).to_broadcast([128, nw, 128])
                        nc.vector.tensor_tensor(
                            out=oh[:, 0 : nw * 128].rearrange(
                                "p (w c) -> p w c", c=128),
                            in0=iota_b, in1=tl_b, op=mybir.AluOpType.is_equal,
                        )
                        psw = pswp.tile([128, 136], F32, tag="ps", name=f"ps{w}")
                        for k in range(nw):
                            nc.tensor.matmul(
                                out=psw[:], lhsT=oh[:, k * 128 : (k + 1) * 128],
                                rhs=wt[:, k, :],
                                start=(k == 0), stop=(k == nw - 1),
                            )
                        dn = epi.tile([128, 8], F32, tag="dn", name=f"dn{w}")
                        nc.vector.tensor_scalar_add(out=dn[:], in0=psw[:, 128:136],
                                                    scalar1=1e-16)
                        rc = epi.tile([128, 8], F32, tag="rc", name=f"rc{w}")
                        nc.vector.reciprocal(out=rc[:], in_=dn[:])
                        rc_b = rc[:].rearrange("p (h o) -> p h o", o=1).to_broadcast(
                            [128, 8, 16])
                        nc.vector.tensor_tensor(
                            out=oslice.rearrange("p (h f) -> p h f", f=16),
                            in0=psw[:, 0:HF].rearrange("p (h f) -> p h f", f=16),
                            in1=rc_b, op=mybir.AluOpType.mult,
                        )
                    nc.sync.dma_start(
                        out=outp[:, w0 * 128 : w1 * 128],
                        in_=outstage[:, 0 : (w1 - w0) * 128],
                    )
    nc.compile()
    return nc


def host_prep(trg, src, W_trg, W_src, a_src, a_trg, N, ncores, TPC, WPC, NPAD):
    src2 = np.asarray(src, dtype=np.float32).reshape(-1, D)[:N]
    trg2 = np.asarray(trg, dtype=np.float32).reshape(-1, D)[:N]
    W_src = np.asarray(W_src, dtype=np.float32)
    W_trg = np.asarray(W_trg, dtype=np.float32)
    a_src = np.asarray(a_src, dtype=np.float32)
    a_trg = np.asarray(a_trg, dtype=np.float32)
    w_s = np.einsum("hf,hfd->hd", a_src, W_src.reshape(NH, FOUT, D))
    w_t = np.einsum("hf,hfd->hd", a_trg, W_trg.reshape(NH, FOUT, D))
    wext = np.zeros((128, 144), dtype=np.float32)
    wext[:, 0:HF] = W_src.T
    wext[:, HF : HF + 8] = w_s.T
    wext[:, 136:144] = w_t.T
    bf = ml_dtypes.bfloat16
    srcT = np.zeros((128, NPAD), dtype=np.float32)
    srcT[:, :N] = src2.T
    LROWS = WPC * 128
    trgTls = []
    for c in range(ncores):
        t = np.zeros((128, LROWS), dtype=np.float32)
        t[:, :TPC] = trg2[c * TPC : (c + 1) * TPC].T
        trgTls.append(t.astype(bf))
    iota = np.tile(np.arange(128, dtype=np.float32), (128, 1))
    return srcT.astype(bf), trgTls, wext.astype(bf), iota.astype(bf)


_CACHE = {}


def run_graph(trg, src, edge_index, W_trg, W_src, a_src, a_trg, N, ncores,
              trace=False):
    global LAST_EXEC_NS
    si = np.asarray(edge_index[0], dtype=np.int64)
    ti = np.asarray(edge_index[1], dtype=np.int64)
    sched, per_core = build_schedule(si, ti, N, ncores)
    TPC, WPC, NPAD = sched["TPC"], sched["WPC"], sched["NPAD"]
    T_total, idx_cols = sched["T_total"], sched["idx_cols"]

    srcT, trgTls, wext, iota = host_prep(
        trg, src, W_trg, W_src, a_src, a_trg, N, ncores, TPC, WPC, NPAD
    )

    key = (N, ncores, tuple(sched["NT"].ravel().tolist()))
    if key not in _CACHE:
        _CACHE[key] = build_nc(sched)
    nc = _CACHE[key]

    in_maps = []
    for c in range(ncores):
        idx1, tl2, tlrep = per_core[c]
        i1 = np.zeros((128, max(idx_cols, 8)), dtype=np.int16)
        i1[:, : idx1.shape[1]] = idx1
        t2 = np.full((128, max(T_total, 1)), 255.0, dtype=ml_dtypes.bfloat16)
        t2[:, : tl2.shape[1]] = tl2
        trp = np.zeros((128, max(T_total * 128, 128)), dtype=ml_dtypes.bfloat16)
        trp[:, : tlrep.shape[1]] = tlrep
        in_maps.append(
            {"srcT": srcT, "trgTl": trgTls[c], "wext": wext, "iota": iota,
             "idx1": i1, "tl2": t2, "tlrep": trp,
             "iotac": np.arange(128, dtype=np.float32).astype(ml_dtypes.bfloat16).reshape(128, 1)}
        )

    if trace:
        trace = _install_trace_shim()
    res = run_bass_kernel_spmd(nc, in_maps, core_ids=list(range(ncores)), trace=trace)
    LAST_EXEC_NS = res.exec_time_ns
    out = np.zeros((N, HF), dtype=np.float32)
    for c in range(ncores):
        o = res.results[c]["out"]  # [128, WPC*128]
        o = o.reshape(128, WPC, 128).transpose(1, 0, 2).reshape(WPC * 128, 128)
        out[c * TPC : (c + 1) * TPC] = o[:TPC]
    return out


def kernel(trg, src, edge_index, W_trg, W_src, a_src, a_trg):
    N = 100000
    out = run_graph(trg, src, edge_index, W_trg, W_src, a_src, a_trg, N, 8,
                    trace=bool(os.environ.get("KERNEL_TRACE")))
    return out.reshape(1, N, HF)
